# revision 20
# baseline (speedup 1.0000x reference)
"""Multi-head attention (B=4, S=2048, D=1024, H=16) on 8 TRN2 NeuronCores.

Sharding: no collectives. Core c handles batch b = c//2, query-half qh = c%2
(1024 query rows). K/V projections for the batch are computed on both cores of
the pair (25% duplicated projection FLOPs, zero communication).

Math (per core), all in a "transposed" feature-major layout so softmax sums
land on free-dim columns and every operand feeds the PE without transposes:
  QT[n, q]  = (WqT tiles).T @ xT        (+ b_q per-partition via ACT bias)
  KT[n, k]  = (WkT tiles).T @ xT        (b_k provably cancels in softmax)
  V [k, n]  = (xT tiles).T @ WvT        (+ b_v via rank-1 ones matmul)
  sT[k, q]  = KT_h.T @ QT_h             (contraction d_k=64)
  eT        = exp(sT / 8)               (ACT, no max-subtraction: |s/8| < ~2.5)
  sum[q]    = ones.T @ eT               (M=1 matmul, col-packed per head pair)
  cT[d, q]  = V_h.T @ eT                (col-packed pair -> psum partitions 0-63/64-127)
  cT_norm   = cT * broadcast(1/sum)     (gpsimd partition_broadcast + DVE mul)
  out[q, n] = (cT tiles).T @ WoT + b_o  (rank-1 ones matmul for bias)

Inputs are rounded to bf16 on the host (weights/x pre-transposed); accumulation
is fp32 in PSUM. The per-core xT has its own query-half swapped to columns
0..1023 so all 8 cores run one SPMD graph (a consistent permutation of the
key/value sequence axis is a softmax no-op).
"""

import numpy as np
import ml_dtypes

BF16 = ml_dtypes.bfloat16

D = 1024      # d_model
S = 2048      # sequence length
QL = 1024     # query rows per core (half a batch)
H = 16        # heads
DK = 64       # head dim
NT = D // 128   # 8  d_model tiles
ST = S // 128   # 16 sequence tiles
QB = QL // 512  # 2  query blocks of 512

_NC_CACHE = {}


def _build_nc():
    if "nc" in _NC_CACHE:
        return _NC_CACHE["nc"]

    import concourse.bass as bass
    import concourse.mybir as mybir
    import concourse.tile as tile
    from concourse import bacc

    f32 = mybir.dt.float32
    bf16 = mybir.dt.bfloat16
    AFT = mybir.ActivationFunctionType

    # Bacc (not raw Bass): its compile() pass splits multi-wait instructions
    # into event semaphores (walrus allows one sync wait per instruction),
    # inserts gpsimd library loads, and lowers custom ISA instructions.
    nc = bacc.Bacc(name="mha8")

    xt_d = nc.dram_tensor("xt", [D, S], bf16, kind="ExternalInput")
    wqt_d = nc.dram_tensor("wqt", [D, D], bf16, kind="ExternalInput")
    wkt_d = nc.dram_tensor("wkt", [D, D], bf16, kind="ExternalInput")
    wvt_d = nc.dram_tensor("wvt", [D, D], bf16, kind="ExternalInput")
    wot_d = nc.dram_tensor("wot", [D, D], bf16, kind="ExternalInput")
    bq_d = nc.dram_tensor("bq", [128, NT], f32, kind="ExternalInput")
    bvt_d = nc.dram_tensor("bvt", [1, D], bf16, kind="ExternalInput")
    bot_d = nc.dram_tensor("bot", [1, D], bf16, kind="ExternalInput")
    out_d = nc.dram_tensor("out", [QL, D], f32, kind="ExternalOutput")

    with tile.TileContext(nc) as tc:
        with (
            tc.tile_pool(name="persist", bufs=1) as persist,
            tc.tile_pool(name="small", bufs=2) as small,
            tc.tile_pool(name="misc512", bufs=4) as misc512,
        ):
            # ---- persistent SBUF ----
            qt_sb = persist.tile([128, NT, QL], bf16)    # QT: feature-major Q
            kt_sb = persist.tile([128, NT, S], bf16)     # KT: feature-major K
            vp_sb = persist.tile([128, ST, D], bf16)     # V natural [k, n]
            ctx_sb = persist.tile([128, NT, QL], bf16)   # normalized context.T
            bq_sb = persist.tile([128, NT], f32)
            bvt_sb = persist.tile([1, D], bf16)
            bot_sb = persist.tile([1, D], bf16)
            ones_sb = persist.tile([128, 1], bf16)   # lhsT for sum matmuls (K=128, M=1)
            nc.vector.memset(ones_sb, 1.0)
            ones_row = persist.tile([1, 128], bf16)  # lhsT for rank-1 bias matmuls
            nc.vector.memset(ones_row, 1.0)

            nc.sync.dma_start(out=bq_sb, in_=bq_d[:, :])
            nc.sync.dma_start(out=bvt_sb, in_=bvt_d[:, :])
            nc.sync.dma_start(out=bot_sb, in_=bot_d[:, :])

            # ================= phase 1: projections =================
            with (
                tc.tile_pool(name="ph1w", bufs=1) as ph1w,
                tc.tile_pool(name="ps1", bufs=4, space="PSUM") as ps1,
            ):
                xt_sb = ph1w.tile([128, NT, S], bf16)
                wqt_sb = ph1w.tile([128, NT, D], bf16)
                wkt_sb = ph1w.tile([128, NT, D], bf16)
                wvt_sb = ph1w.tile([128, NT, D], bf16)

                nc.sync.dma_start(out=xt_sb, in_=xt_d[:, :].rearrange("(t p) s -> p t s", p=128))
                nc.sync.dma_start(out=wqt_sb, in_=wqt_d[:, :].rearrange("(t p) n -> p t n", p=128))
                nc.sync.dma_start(out=wkt_sb, in_=wkt_d[:, :].rearrange("(t p) n -> p t n", p=128))
                nc.sync.dma_start(out=wvt_sb, in_=wvt_d[:, :].rearrange("(t p) n -> p t n", p=128))

                # QT[n, q]: lhsT = WqT d-tile slice, rhs = xT (query half = cols 0..QL)
                for i in range(NT):
                    for jq in range(QB):
                        ps = ps1.tile([128, 512], f32, tag="ps")
                        for k in range(NT):
                            nc.tensor.matmul(
                                ps,
                                wqt_sb[:, k, i * 128:(i + 1) * 128],
                                xt_sb[:, k, jq * 512:(jq + 1) * 512],
                                start=(k == 0),
                                stop=(k == NT - 1),
                            )
                        nc.scalar.activation(
                            out=qt_sb[:, i, jq * 512:(jq + 1) * 512],
                            in_=ps,
                            func=AFT.Identity,
                            bias=bq_sb[:, i:i + 1],
                            scale=1.0,
                        )

                # KT[n, k_seq]: full sequence, no bias (b_k cancels in softmax)
                for i in range(NT):
                    for jk in range(S // 512):
                        ps = ps1.tile([128, 512], f32, tag="ps")
                        for k in range(NT):
                            nc.tensor.matmul(
                                ps,
                                wkt_sb[:, k, i * 128:(i + 1) * 128],
                                xt_sb[:, k, jk * 512:(jk + 1) * 512],
                                start=(k == 0),
                                stop=(k == NT - 1),
                            )
                        nc.vector.tensor_copy(
                            out=kt_sb[:, i, jk * 512:(jk + 1) * 512], in_=ps
                        )

                # V natural [k_seq, n]: lhsT = xT seq-slice, rhs = WvT; + ones x b_v
                for m in range(ST):
                    for jn in range(D // 512):
                        ps = ps1.tile([128, 512], f32, tag="ps")
                        for k in range(NT):
                            nc.tensor.matmul(
                                ps,
                                xt_sb[:, k, m * 128:(m + 1) * 128],
                                wvt_sb[:, k, jn * 512:(jn + 1) * 512],
                                start=(k == 0),
                                stop=False,
                            )
                        nc.tensor.matmul(
                            ps,
                            ones_row,
                            bvt_sb[:, jn * 512:(jn + 1) * 512],
                            start=False,
                            stop=True,
                        )
                        nc.vector.tensor_copy(
                            out=vp_sb[:, m, jn * 512:(jn + 1) * 512], in_=ps
                        )

            # ===== pool spanning phases 2+3: W_o tiles (DMA hidden under phase 2) =====
            from contextlib import ExitStack
            late_ctx = ExitStack()
            late = late_ctx.enter_context(tc.tile_pool(name="late", bufs=1))
            wot_sb = late.tile([128, NT, D], bf16)
            nc.sync.dma_start(out=wot_sb, in_=wot_d[:, :].rearrange("(t p) n -> p t n", p=128))

            # ================= phase 2: attention =================
            with (
                tc.tile_pool(name="expp", bufs=2) as expp,
                tc.tile_pool(name="ps_sc", bufs=2, space="PSUM") as ps_sc,
                tc.tile_pool(name="ps_ctx", bufs=2, space="PSUM") as ps_ctx,
                tc.tile_pool(name="ps_sum", bufs=2, space="PSUM") as ps_sum,
                tc.tile_pool(name="dramp", bufs=4, space="DRAM") as dramp,
            ):
                for j in range(H // 2):  # head pair (2j, 2j+1)
                    et = [None, None]
                    for hh in range(2):
                        h = 2 * j + hh
                        pb = 64 * hh  # partition base of head's features in tile j
                        e_t = expp.tile([128, ST, QL], bf16, tag="e_t")
                        et[hh] = e_t
                        for kt in range(ST):
                            ps_s = ps_sc.tile([128, QL], f32, tag="ps_s")
                            for jq in range(QB):
                                nc.tensor.matmul(
                                    ps_s[:, jq * 512:(jq + 1) * 512],
                                    kt_sb[pb:pb + 64, j, kt * 128:(kt + 1) * 128],
                                    qt_sb[pb:pb + 64, j, jq * 512:(jq + 1) * 512],
                                    start=True,
                                    stop=True,
                                )
                            nc.scalar.activation(
                                out=e_t[:, kt, :],
                                in_=ps_s,
                                func=AFT.Exp,
                                scale=0.125,
                            )

                    for jq in range(QB):
                        qs = slice(jq * 512, (jq + 1) * 512)
                        ps_c = ps_ctx.tile([128, 512], f32, tag="ps_c")
                        ps_m = ps_sum.tile([128, 512], f32, tag="ps_m")
                        for hh in range(2):
                            h = 2 * j + hh
                            pb = 64 * hh
                            for kt in range(ST):
                                # context.T: head hh -> psum partitions pb..pb+64
                                nc.tensor.matmul(
                                    ps_c[pb:pb + 64, :],
                                    vp_sb[:, kt, h * 64:(h + 1) * 64],
                                    et[hh][:, kt, qs],
                                    start=(kt == 0),
                                    stop=(kt == ST - 1),
                                    tile_position=(0, pb),
                                )
                                # softmax denominator -> psum partition pb
                                nc.tensor.matmul(
                                    ps_m[pb:pb + 1, :],
                                    ones_sb,
                                    et[hh][:, kt, qs],
                                    start=(kt == 0),
                                    stop=(kt == ST - 1),
                                    tile_position=(0, pb),
                                )

                        recip = small.tile([128, 512], f32, tag="recip")
                        rb = misc512.tile([128, 512], f32, tag="rb")
                        for hh in range(2):
                            h = 2 * j + hh
                            pb = 64 * hh
                            nc.vector.reciprocal(
                                out=recip[pb:pb + 1, :], in_=ps_m[pb:pb + 1, :]
                            )
                            rd = dramp.tile([1, 512], f32, tag="rd")
                            nc.sync.dma_start(out=rd, in_=recip[pb:pb + 1, :])
                            src_b = bass.AP(
                                tensor=rd.tensor,
                                offset=rd.offset,
                                ap=[[0, 64]] + [list(a) for a in rd.ap[1:]],
                            )
                            nc.sync.dma_start(out=rb[pb:pb + 64, :], in_=src_b)
                        nc.vector.tensor_mul(ctx_sb[:, j, qs], ps_c, rb)

            # ================= phase 3: output projection =================
            with tc.tile_pool(name="ps3", bufs=4, space="PSUM") as ps3:
                for qt in range(QL // 128):
                    for jn in range(D // 512):
                        ps = ps3.tile([128, 512], f32, tag="ps")
                        for k in range(NT):
                            nc.tensor.matmul(
                                ps,
                                ctx_sb[:, k, qt * 128:(qt + 1) * 128],
                                wot_sb[:, k, jn * 512:(jn + 1) * 512],
                                start=(k == 0),
                                stop=False,
                            )
                        nc.tensor.matmul(
                            ps,
                            ones_row,
                            bot_sb[:, jn * 512:(jn + 1) * 512],
                            start=False,
                            stop=True,
                        )
                        o_sb = misc512.tile([128, 512], f32, tag="o_sb")
                        nc.vector.tensor_copy(out=o_sb, in_=ps)
                        nc.sync.dma_start(
                            out=out_d[qt * 128:(qt + 1) * 128, jn * 512:(jn + 1) * 512],
                            in_=o_sb,
                        )
            late_ctx.close()

    nc.finalize()
    _NC_CACHE["nc"] = nc
    return nc


def _prep_in_maps(x, W_q, b_q, W_k, W_v, b_v, W_o, b_o):
    wqt = np.ascontiguousarray(W_q.T).astype(BF16)
    wkt = np.ascontiguousarray(W_k.T).astype(BF16)
    wvt = np.ascontiguousarray(W_v.T).astype(BF16)
    wot = np.ascontiguousarray(W_o.T).astype(BF16)
    bq = np.ascontiguousarray(b_q.reshape(NT, 128).T).astype(np.float32)
    bvt = b_v.reshape(1, D).astype(BF16)
    bot = b_o.reshape(1, D).astype(BF16)

    in_maps = []
    for c in range(8):
        b, qh = divmod(c, 2)
        xT = x[b].T  # [D, S]
        if qh == 0:
            xt = xT
        else:
            xt = np.concatenate([xT[:, QL:], xT[:, :QL]], axis=1)
        xt = np.ascontiguousarray(xt).astype(BF16)
        in_maps.append(
            {
                "xt": xt,
                "wqt": wqt, "wkt": wkt, "wvt": wvt, "wot": wot,
                "bq": bq, "bvt": bvt, "bot": bot,
            }
        )
    return in_maps


def _run(inputs, trace=False, trace_kwargs=None):
    from concourse import bass_utils

    nc = _build_nc()
    in_maps = _prep_in_maps(
        inputs["x"], inputs["W_q"], inputs["b_q"], inputs["W_k"],
        inputs["W_v"], inputs["b_v"], inputs["W_o"], inputs["b_o"],
    )
    kwargs = {}
    if trace:
        kwargs["trace"] = True
        if trace_kwargs:
            kwargs.update(trace_kwargs)
    res = bass_utils.run_bass_kernel_spmd(
        nc, in_maps, core_ids=list(range(8)), **kwargs
    )
    out = np.empty((4, S, D), np.float32)
    for c, r in enumerate(res.results):
        b, qh = divmod(c, 2)
        out[b, qh * QL:(qh + 1) * QL, :] = r["out"]
    return out, res


def kernel(**inputs):
    out, _ = _run(inputs, trace=False)
    return out
